# revision 23
# baseline (speedup 1.0000x reference)
"""Causal self-attention (B=4, S=2048, D=768, H=12) on 8 trn2 NeuronCores.

Sharding (Megatron-style): DP over the 4 batches x TP=2 over heads.
Core c handles batch c//2 with heads (c%2)*6 .. +6: qkv_proj column-parallel,
out_proj row-parallel; the TP pair's partial outputs are summed on the host.

Per-core kernel (all matmuls fp32r, fp32 data widths):
  A. stream x [2048,768], PE-transpose to xT [d(part), s]
  B. qkT = (x @ Wqk)^T directly in [feat(part), s] layout (W stationary,
     xT moving); V in natural [s(part), feat] layout with a ones column
     appended (V') so the PV matmul also produces the softmax denominator.
     Score scale 1/sqrt(64) and qkv bias are folded in (scale on host into
     Wq/bq; bias added during the PSUM->SBUF copy, per-partition in the
     transposed layout).
  C. flash-style causal attention per head: S^T tile = K_tile @ Q^T
     (contraction = head dim 64), exp on ACT batched 2 k-tiles per
     ACTIVATE (amortizes the 352-cycle fixed cost; gap columns of
     diagonal groups hold garbage exp values that no PV matmul reads),
     causal via narrowed matmuls + one 128x128 mask multiply (GPSIMD)
     per diagonal block; O^T accumulated in PSUM over k tiles via
     lhsT=V' (no max subtraction: scores are O(5), fp32 exp is safe);
     denominator row broadcast across partitions with a K=1 ones matmul,
     applied on the PSUM->SBUF copy.
  D. out_partial = O @ Wout_slice via lhsT=OT chunks, written [s, 768].
"""
import sys
import numpy as np
import concourse.bass as bass
import concourse.mybir as mybir
import concourse.tile as tile
from concourse import bacc
from concourse.bass_utils import run_bass_kernel_spmd
from concourse.masks import make_identity

B, S, D = 4, 2048, 768
H, HD = 12, 64
N_CORES = 8
HPC = H // 2          # heads per core = 6
FQK = HPC * HD        # 384 features per core for each of q,k,v
F32 = mybir.dt.float32
F32R = mybir.dt.float32r

N_ST = S // 128       # 16 s tiles
N_QC = S // 512       # 4 q chunks
N_DT = D // 128       # 6 d_model tiles

TRACE = False         # set by test.py for profiling runs
_CACHE = {}
PHASE_MARKS = []      # (phase_name, first_inst_id) — filled during _emit


def _mark(nc, name):
    PHASE_MARKS.append((name, nc.next_id()))


def _emit(nc):
    x_d = nc.dram_tensor("x", [S, D], F32, kind="ExternalInput").ap()
    wqkv_d = nc.dram_tensor("wqkv", [D, 3 * FQK], F32R, kind="ExternalInput").ap()
    bqk_d = nc.dram_tensor("bqk", [128, 6], F32, kind="ExternalInput").ap()
    vb_d = nc.dram_tensor("vb", [128, FQK], F32, kind="ExternalInput").ap()
    wout_d = nc.dram_tensor("wout", [FQK, D], F32R, kind="ExternalInput").ap()
    out_d = nc.dram_tensor("out", [S, D], F32, kind="ExternalOutput").ap()

    with tile.TileContext(nc) as tc:
        with tc.tile_pool(name="const", bufs=1) as pc, \
             tc.tile_pool(name="qkT", bufs=1) as pqk, \
             tc.tile_pool(name="vn", bufs=1) as pvn, \
             tc.tile_pool(name="wstr", bufs=3) as pw, \
             tc.tile_pool(name="xstr", bufs=3) as px, \
             tc.tile_pool(name="pt", bufs=4) as ppt, \
             tc.tile_pool(name="ep", bufs=2) as pep, \
             tc.tile_pool(name="oraw", bufs=8) as por, \
             tc.tile_pool(name="outp", bufs=2) as pout, \
             tc.tile_pool(name="ps", bufs=3, space="PSUM") as pp, \
             tc.tile_pool(name="pso", bufs=2, space="PSUM") as ppo:

            ident = pc.tile([128, 128], F32)
            make_identity(nc, ident)
            # causal block mask: keep where local q (free) >= local k (part)
            mask = pc.tile([128, 128], F32)
            nc.gpsimd.memset(mask[:], 1.0)
            nc.gpsimd.affine_select(
                out=mask[:], in_=mask[:], compare_op=mybir.AluOpType.is_ge,
                fill=0.0, base=0, channel_multiplier=-1, pattern=[[1, 128]])
            ones_r = pc.tile([1, 64], F32R)
            nc.vector.memset(ones_r[:].bitcast(F32), 1.0)
            bqk_sb = pc.tile([128, 6], F32)
            nc.sync.dma_start(bqk_sb[:], bqk_d[:])
            vb_sb = pc.tile([128, FQK], F32)
            nc.sync.dma_start(vb_sb[:], vb_d[:])
            wv_sb = pc.tile([128, N_DT, FQK], F32R)
            wout_sb = pc.tile([128, FQK // 128, D], F32R)

            # Vn: [s(part), s_tile, head, 65] with ones col at 64
            vn = pvn.tile([128, N_ST, HPC, HD + 1], F32R)
            # qkT: [feat%128(part), f_tile (0-2 q | 3-5 k), s]
            qkT = pqk.tile([128, 6, S], F32R)

            # ---- attention k-loop for one (head, q-chunk): returns the
            #      un-normalized O^T+den in SBUF (normalization deferred) ----
            def attn_kloop(h, qc):
                po = (h % 2) * 64
                qt = h // 2         # q f_tile
                kt_f = 3 + h // 2   # k f_tile
                ps_o = ppo.tile([128, 512], F32, tag="o")
                n_kt = 4 * (qc + 1)
                for ktg in range(0, n_kt, 2):
                    ps_s = pp.tile([128, 1024], F32, tag="s")
                    offs = []
                    for j in range(2):
                        kt = ktg + j
                        q_off = max(0, kt * 128 - qc * 512)
                        offs.append(q_off)
                        nc.tensor.matmul(
                            ps_s[:, j * 512 + q_off:(j + 1) * 512],
                            qkT[po:po + 64, kt_f, kt * 128:(kt + 1) * 128],
                            qkT[po:po + 64, qt,
                                qc * 512 + q_off:(qc + 1) * 512],
                            start=True, stop=True)
                    pt = ppt.tile([128, 1024], F32R, tag="pt")
                    nc.scalar.activation(
                        pt[:, offs[0]:], ps_s[:, offs[0]:],
                        mybir.ActivationFunctionType.Exp)
                    for j in range(2):
                        kt = ktg + j
                        q_off = offs[j]
                        if kt * 128 >= qc * 512:  # diagonal block
                            sl = slice(j * 512 + q_off, j * 512 + q_off + 128)
                            nc.gpsimd.tensor_tensor(
                                pt[:, sl], pt[:, sl], mask[:],
                                mybir.AluOpType.mult)
                        nc.tensor.matmul(
                            ps_o[0:HD + 1, q_off:], vn[:, kt, h, :],
                            pt[:, j * 512 + q_off:(j + 1) * 512],
                            start=(kt == 0), stop=(kt == n_kt - 1))
                oraw = por.tile([65, 512], F32, tag="oraw")
                nc.vector.tensor_copy(oraw[:], ps_o[0:HD + 1, :])
                return oraw

            with tc.tile_pool(name="xT", bufs=1) as pxt:
                xT = pxt.tile([128, N_DT, S], F32R)

                _mark(nc, "A:transpose")
                # ---- phase A: stream x, transpose to xT ----
                for st in range(N_ST):
                    x_t = px.tile([128, D], F32, tag="x")
                    nc.sync.dma_start(x_t[:], x_d[st * 128:(st + 1) * 128, :])
                    for half, ndc in ((0, 4), (1, 2)):
                        ps_t = ppo.tile([128, 512], F32, tag="o")
                        for i in range(ndc):
                            dc = half * 4 + i
                            nc.tensor.transpose(
                                ps_t[:, i * 128:(i + 1) * 128],
                                x_t[:, dc * 128:(dc + 1) * 128], ident[:])
                        nc.scalar.copy(
                            xT[:, half * 4:half * 4 + ndc,
                               st * 128:(st + 1) * 128],
                            ps_t[:, :ndc * 128].rearrange(
                                "p (t s) -> p t s", s=128))

                # weight DMAs after the x stream so x wins the DMA queue
                nc.sync.dma_start(
                    wv_sb[:],
                    wqkv_d[:, 2 * FQK:].rearrange("(t p) f -> p t f", p=128))
                nc.sync.dma_start(
                    wout_sb[:], wout_d.rearrange("(t p) o -> p t o", p=128))

                _mark(nc, "B:qkv")
                # ---- phase B: V' first, then qkT f_tiles paired per head,
                #      with qc0 attention interleaved as its inputs land ----
                nc.vector.memset(vn[:].bitcast(F32), 1.0)
                vb_h = vb_sb.rearrange("p (h d) -> p h d", d=HD)
                for st2 in range(N_ST // 2):
                    ps_v = pp.tile([128, 1024], F32, tag="s")
                    for j in range(2):
                        st = 2 * st2 + j
                        for dc in range(N_DT):
                            nc.tensor.matmul(
                                ps_v[:, j * 512:j * 512 + FQK],
                                xT[:, dc, st * 128:(st + 1) * 128],
                                wv_sb[:, dc, :],
                                start=(dc == 0), stop=(dc == N_DT - 1))
                    for j in range(2):
                        st = 2 * st2 + j
                        nc.vector.tensor_tensor(
                            vn[:, st, :, 0:HD],
                            ps_v[:, j * 512:j * 512 + FQK].rearrange(
                                "p (h d) -> p h d", d=HD),
                            vb_h, mybir.AluOpType.add)

                oraw0 = {}
                for pi in range(3):
                    for ft in (pi, 3 + pi):
                        w_t = pw.tile([128, N_DT, 128], F32R, tag="w")
                        nc.sync.dma_start(
                            w_t[:],
                            wqkv_d[:, ft * 128:(ft + 1) * 128].rearrange(
                                "(t p) f -> p t f", p=128))
                        for sc2 in range(2):
                            ps_qk = pp.tile([128, 1024], F32, tag="s")
                            for j in range(2):
                                sc = 2 * sc2 + j
                                for dc in range(N_DT):
                                    nc.tensor.matmul(
                                        ps_qk[:, j * 512:(j + 1) * 512],
                                        w_t[:, dc, :],
                                        xT[:, dc, sc * 512:(sc + 1) * 512],
                                        start=(dc == 0),
                                        stop=(dc == N_DT - 1))
                            nc.scalar.activation(
                                qkT[:, ft, sc2 * 1024:(sc2 + 1) * 1024],
                                ps_qk[:],
                                mybir.ActivationFunctionType.Identity,
                                bias=bqk_sb[:, ft:ft + 1])
                    # heads 2*pi, 2*pi+1 have q+k f_tiles now: start qc0
                    for h in (2 * pi, 2 * pi + 1):
                        oraw0[h] = attn_kloop(h, 0)

            # xT pool closed; OT reuses its space
            with tc.tile_pool(name="OT", bufs=1) as pot:
                oT = pot.tile([128, FQK // 128, S], F32R)

                def normalize(h, qc, oraw):
                    po = (h % 2) * 64
                    recip = pep.tile([1, 512], F32R, tag="recip")
                    with nc.allow_low_precision(reason="fp32-width recip"):
                        nc.vector.reciprocal(recip[:], oraw[HD:HD + 1, :])
                    rb = pep.tile([64, 512], F32R, tag="rb")
                    nc.gpsimd.partition_broadcast(rb[:], recip[:])
                    nc.vector.tensor_tensor(
                        oT[po:po + 64, h // 2, qc * 512:(qc + 1) * 512],
                        oraw[0:HD, :], rb[:], mybir.AluOpType.mult)

                def emit_proj(st):
                    o_sb = pout.tile([128, D], F32, tag="o_sb")
                    ps_d = pp.tile([128, 1024], F32, tag="s")
                    for oc in range(2):
                        for ht in range(FQK // 128):
                            nc.tensor.matmul(
                                ps_d[:, oc * 512:oc * 512 + 384],
                                oT[:, ht, st * 128:(st + 1) * 128],
                                wout_sb[:, ht, oc * 384:(oc + 1) * 384],
                                start=(ht == 0),
                                stop=(ht == FQK // 128 - 1))
                    nc.vector.tensor_copy(
                        o_sb.rearrange("p (j x) -> p j x", x=384),
                        ps_d[:].rearrange(
                            "p (j y) -> p j y", y=512)[:, :, :384])
                    nc.sync.dma_start(
                        out_d[st * 128:(st + 1) * 128, :], o_sb[:])

                # previous q-chunk's normalization + out-proj interleave
                # into the next q-chunk's head loop (progressive slot reuse)
                prev = {0: oraw0}
                for qc in range(1, N_QC):
                    _mark(nc, f"C:attn qc={qc}")
                    cur = {}
                    po_ = prev[qc - 1]
                    for h in range(HPC):
                        if h < 3:
                            normalize(2 * h, qc - 1, po_[2 * h])
                            normalize(2 * h + 1, qc - 1, po_[2 * h + 1])
                        else:
                            emit_proj((qc - 1) * 4 + (h - 3))
                        cur[h] = attn_kloop(h, qc)
                    emit_proj((qc - 1) * 4 + 3)
                    prev[qc] = cur

                _mark(nc, "D:tail")
                for h in range(HPC):
                    normalize(h, N_QC - 1, prev[N_QC - 1][h])
                for st in range(12, 16):
                    emit_proj(st)


def _build():
    if "nc" not in _CACHE:
        nc = bacc.Bacc("TRN2", target_bir_lowering=False, debug=False,
                       num_devices=N_CORES)
        _emit(nc)
        nc.compile()
        _CACHE["nc"] = nc
    return _CACHE["nc"]


def kernel(x, qkv_w, qkv_b, out_w, out_b):
    x = np.ascontiguousarray(np.asarray(x, dtype=np.float32))
    qkv_w = np.asarray(qkv_w, dtype=np.float32)
    qkv_b = np.asarray(qkv_b, dtype=np.float32)
    out_w = np.asarray(out_w, dtype=np.float32)
    out_b = np.asarray(out_b, dtype=np.float32)

    nc = _build()
    scale = HD ** -0.5
    in_maps = []
    for c in range(N_CORES):
        b, half = c // 2, c % 2
        fq = slice(half * FQK, (half + 1) * FQK)
        fk = slice(D + half * FQK, D + (half + 1) * FQK)
        fv = slice(2 * D + half * FQK, 2 * D + (half + 1) * FQK)
        wq = qkv_w[:, fq] * scale
        wk = qkv_w[:, fk]
        wv = qkv_w[:, fv]
        wqkv = np.ascontiguousarray(
            np.concatenate([wq, wk, wv], axis=1), dtype=np.float32)
        bqk = np.concatenate([qkv_b[fq] * scale, qkv_b[fk]])  # [768]
        bqk = np.ascontiguousarray(
            bqk.reshape(6, 128).T, dtype=np.float32)          # [128, 6]
        vb = np.ascontiguousarray(
            np.broadcast_to(qkv_b[fv], (128, FQK)), dtype=np.float32)
        wout = np.ascontiguousarray(
            out_w[half * FQK:(half + 1) * FQK, :], dtype=np.float32)
        in_maps.append({
            "x": np.ascontiguousarray(x[b]),
            "wqkv": wqkv, "bqk": bqk, "vb": vb, "wout": wout,
        })

    res = run_bass_kernel_spmd(nc, in_maps, list(range(N_CORES)), trace=TRACE)
    parts = [res.results[c]["out"] for c in range(N_CORES)]
    out = np.empty((B, S, D), dtype=np.float32)
    for b in range(B):
        out[b] = parts[2 * b] + parts[2 * b + 1] + out_b
    if TRACE:
        kernel.last_results = res
    return out
